# revision 11
# baseline (speedup 1.0000x reference)
"""Delta-modulator scan kernel for Trainium2 — raw bass (no Tile).

Per (b, r): sequential scan over the first 232 columns of x[.,.,252] with
state (dc, delta, signed run-counter); outputs UP[232] | DN[232] | x[232:252]
-> out [., ., 484] f32. Data-parallel over batch: 16 batches/core, 8 cores;
per-core 16384 instances laid out as [128 partitions x 128 free].

Structure (timings per the TimelineSim cost model; 251.0us/core total):
- The whole scan runs on the DVE back-to-back with NO semaphores inside the
  loop (in-order engine; full-width streaming ops make same-engine RAW safe):
  y = x - dc; v = (y>dl)-(y<-dl) written into xv slot t (in place over the
  consumed x column); copy_predicated dc-latch; run-counter; delta update.
  5 ops x ~194ns x 232 steps ~= 225us. Tile's per-dependency semaphore
  chains would add ~190ns/step — that is why this is raw bass.
- Warm start: the first 16 columns load as two f-half DMAs and steps 0..4
  run per-half at half width, filling DVE idle time while loads land.
- Remaining loads: [16:32) at the descriptor floor, then [32:160) and
  [104:232) as 128-column transfers whose 512B contiguous runs avoid the
  <512B 2x DMA penalty ([104:160) is rewritten with identical data).
- up/dn are extracted from the v history by the otherwise-idle Activation
  engine (relu(v), relu(-v)) into small staging tiles, in column chunks
  sized to minimize max_k(chunk readiness + remaining store mass): 32-col
  early, 24/21/18-col late, 17-col last (floor-rate pair). Extractions are
  split into pieces so each store fires as soon as its last column lands;
  column 231's up/dn are finished on the DVE right after the last DM_V
  (guarded against the staging-column reuse by chunk 7's stores).
- Tail passthrough out[464:484) = x[232:252) as a direct DRAM->DRAM DMA.
- Manual semaphores: dma_sem (+16 per DMA, FIFO order), dve_sem (+1 per
  step via copy_predicated, +1 by the init memset, +1 each by the final
  DM_V/DM_UP/DM_DN), act_sem (+1 per extraction piece).
"""

import os
from contextlib import ExitStack

import numpy as np

import concourse.bass as bass
from concourse import bacc, mybir
from concourse.bass_utils import run_bass_kernel_spmd
import concourse.dve_ops as dve_ops_mod
from concourse.dve_spec import (
    Spec, Src0, Src1, C0, C1, C2, Zero, One, maxx, minn, select, lower,
)
from concourse.dve_spec import _has_src1
from concourse.dve_uop import DveOpSpec

AluOp = mybir.AluOpType
F32 = mybir.dt.float32


def _register_op(name: str, spec: Spec) -> "dve_ops_mod.DveOp":
    """Register a custom DVE op at runtime (compute + pin its uop sha)."""
    for existing in dve_ops_mod.OPS:
        if existing.name == name:
            return existing
    opcode = dve_ops_mod._CUSTOM_DVE_ROW_BASE + len(dve_ops_mod.OPS)
    assert opcode < 0x20
    shas = {}
    for ver in ("v3",):
        tmp = DveOpSpec(
            name=name, opcode=opcode, uops=lower(spec, ver=ver), rd1_en=_has_src1(spec)
        )
        shas[ver] = tmp.sha(ver)
    op = dve_ops_mod.DveOp(name, spec, subdim=False, uops_sha=shas)
    dve_ops_mod.OPS.append(op)
    dve_ops_mod._SUB_OPCODE_FOR_NAME[name] = opcode
    dve_ops_mod.CUSTOM_DVE_SPECS[name] = spec
    return op


# cc' = trig ? max(cc,0)+1 : min(cc,0)-1   (in0=cc, in1=v; trig = in1 != 0)
DM_COUNTER = _register_op(
    "DM_COUNTER_ANT",
    Spec(
        body=select(Src1, maxx(Src0, Zero) + One, minn(Src0, Zero) - One),
        reference=lambda in0, in1, s0, s1, imm2: np.where(
            in1 != 0.0, np.maximum(in0, 0) + 1, np.minimum(in0, 0) - 1
        ).astype(np.float32),
    ),
)

# dl' = min(max(dl, (cc<=-3)*0.1), max((cc<3), 0.02))  (in0=cc, in1=dl,
# s0=-3.0, s1=0.1, imm2=0.02)
DM_DELTA = _register_op(
    "DM_DELTA_ANT",
    Spec(
        body=minn(
            maxx(Src1, (Src0 <= C0) * C1),
            maxx(Src0 < (Zero - C0), C2),
        ),
        reference=lambda in0, in1, s0, s1, imm2: np.minimum(
            np.maximum(in1, (in0 <= s0).astype(np.float32) * s1),
            np.maximum((in0 < -s0).astype(np.float32), imm2),
        ).astype(np.float32),
    ),
)

# v = (y > dl) - (y < -dl) in {-1, 0, +1} (never -0.0). Doubles as the
# predication mask (bit pattern nonzero iff trigger).  (in0=y, in1=dl)
DM_V = _register_op(
    "DM_V2_ANT",
    Spec(
        body=(Src0 > Src1) - (Src0 < (Zero - Src1)),
        reference=lambda in0, in1, s0, s1, imm2: (
            (in0 > in1).astype(np.float32) - (in0 < -in1).astype(np.float32)
        ),
    ),
)

# up = max(v, 0); dn = max(-v, 0) — single-input finishers for col 231,
# run on the DVE right after the final DM_V so the closing stores skip
# the Activation-engine round-trip.
DM_UP = _register_op(
    "DM_UP_ANT",
    Spec(
        body=maxx(Src0, Zero),
        reference=lambda in0, in1, s0, s1, imm2: np.maximum(in0, 0).astype(
            np.float32
        ),
    ),
)
DM_DN = _register_op(
    "DM_DN_ANT",
    Spec(
        body=maxx(Zero - Src0, Zero),
        reference=lambda in0, in1, s0, s1, imm2: np.maximum(-in0, 0).astype(
            np.float32
        ),
    ),
)

B, R, C = 128, 1024, 252
NSTEP = 232
NTAIL = C - NSTEP  # 20
OUTC = 2 * NSTEP + NTAIL  # 484
NCORES = 8
BPC = B // NCORES  # 16
INST = BPC * R  # 16384 instances per core
P = 128
F = INST // P  # 128

# store chunking: 32-col early, then 24/20-col late chunks. 20-col pairs
# hit the 7ns/descriptor floor at the same per-column rate as 32-col but
# carry less mass after the scan's last step, which bounds the finish.
CHUNKS = [(0, 32), (32, 32), (64, 32), (96, 32), (128, 24), (152, 24),
          (176, 21), (197, 18), (215, 17)]

_NC_CACHE = {}


def _build_nc() -> bass.Bass:
    key = "nc"
    if key in _NC_CACHE:
        return _NC_CACHE[key]
    nc = bacc.Bacc("TRN2", target_bir_lowering=False, debug=False)
    x = nc.dram_tensor("x", [INST, C], F32, kind="ExternalInput").ap()
    out = nc.dram_tensor("out", [INST, OUTC], F32, kind="ExternalOutput").ap()
    x3 = x.rearrange("(p f) c -> p f c", p=P)  # [128, 128, 252]
    o3 = out.rearrange("(p f) c -> p f c", p=P)  # [128, 128, 484]

    Relu = mybir.ActivationFunctionType.Relu
    NDMA = 6 + 2 * len(CHUNKS)

    def _pieces(k, cn):
        # split each plane's extraction so the store can fire right after
        # the chunk's LAST column's v lands; the final chunk gets a 3-way
        # split so the Act engine is idle when column 231 arrives.
        if k == len(CHUNKS) - 1:
            return [cn - 4, 3]
        return [cn - 1, 1]

    with ExitStack() as ctx:
        # xv slot k holds x_{k-1}; v_t overwrites slot t (x_{t-1} dead).
        xv_t = ctx.enter_context(nc.sbuf_tensor("xv", [P, F, NSTEP + 1], F32))
        s_up_t = ctx.enter_context(nc.sbuf_tensor("s_up", [P, F, 32], F32))
        s_dn_t = ctx.enter_context(nc.sbuf_tensor("s_dn", [P, F, 32], F32))
        dc_t = ctx.enter_context(nc.sbuf_tensor("dc", [P, F], F32))
        dl_ts = [
            ctx.enter_context(nc.sbuf_tensor(f"dl{i}", [P, F], F32))
            for i in range(2)
        ]
        cc_ts = [
            ctx.enter_context(nc.sbuf_tensor(f"cc{i}", [P, F], F32))
            for i in range(2)
        ]
        y_ts = [
            ctx.enter_context(nc.sbuf_tensor(f"y{i}", [P, F], F32))
            for i in range(2)
        ]
        dma_sem = ctx.enter_context(nc.semaphore("dma_sem"))
        dve_sem = ctx.enter_context(nc.semaphore("dve_sem"))
        act_sem = ctx.enter_context(nc.semaphore("act_sem"))

        xv = xv_t.ap()
        s_up = s_up_t.ap()
        s_dn = s_dn_t.ap()
        dc = dc_t.ap()
        dls = [t.ap() for t in dl_ts]
        ccs = [t.ap() for t in cc_ts]
        ys = [t.ap() for t in y_ts]

        with nc.Block() as block:

            @block.sync
            def _(sync_):
                sync = sync_
                # loads: x_k -> slot k+1
                sync.dma_start(
                    xv[:, 0:64, 1:17], x3[:, 0:64, 0:16]
                ).then_inc(dma_sem, 16)
                sync.dma_start(
                    xv[:, 64:128, 1:17], x3[:, 64:128, 0:16]
                ).then_inc(dma_sem, 16)
                sync.dma_start(xv[:, :, 17:33], x3[:, :, 16:32]).then_inc(
                    dma_sem, 16
                )
                sync.dma_start(xv[:, :, 33:161], x3[:, :, 32:160]).then_inc(
                    dma_sem, 16
                )
                sync.dma_start(xv[:, :, 105:233], x3[:, :, 104:232]).then_inc(
                    dma_sem, 16
                )
                # tail passthrough DRAM->DRAM
                sync.dma_start(
                    o3[:, :, 2 * NSTEP : OUTC], x3[:, :, NSTEP:C]
                ).then_inc(dma_sem, 16)
                nact = 0
                last = len(CHUNKS) - 1
                for k, (c0, cn) in enumerate(CHUNKS):
                    nact += len(_pieces(k, cn))
                    sync.wait_ge(act_sem, nact)
                    if k == last:
                        sync.wait_ge(dve_sem, 239)  # DM_UP finisher done
                    sync.dma_start(
                        o3[:, :, c0 : c0 + cn], s_up[:, :, 0:cn]
                    ).then_inc(dma_sem, 16)
                    nact += len(_pieces(k, cn))
                    sync.wait_ge(act_sem, nact)
                    if k == last:
                        sync.wait_ge(dve_sem, 240)  # DM_DN finisher done
                    sync.dma_start(
                        o3[:, :, NSTEP + c0 : NSTEP + c0 + cn], s_dn[:, :, 0:cn]
                    ).then_inc(dma_sem, 16)
                sync.wait_ge(dma_sem, 16 * NDMA)

            @block.vector
            def _(vector):
                vector.memset(dc[:], 0.0)
                vector.memset(dls[0][:], 0.1)
                vector.memset(ccs[0][:], 0.0).then_inc(dve_sem)
                KW = 5  # warm-start steps run per f-half while loads land
                for f0, f1, gate in ((0, 64, 16), (64, 128, 32)):
                    vector.wait_ge(dma_sem, gate)
                    for t in range(KW):
                        xs = xv[:, f0:f1, t + 1]
                        y = ys[t % 2][:, f0:f1]
                        dl = dls[t % 2][:, f0:f1]
                        dl2 = dls[1 - t % 2][:, f0:f1]
                        cc = ccs[t % 2][:, f0:f1]
                        cc2 = ccs[1 - t % 2][:, f0:f1]
                        vslot = xv[:, f0:f1, t]
                        vector.tensor_tensor(
                            y, xs, dc[:, f0:f1], AluOp.subtract
                        )
                        vector._custom_dve(DM_V, out=vslot, in0=y, in1=dl)
                        vector.copy_predicated(
                            dc[:, f0:f1], vslot.bitcast(mybir.dt.int32), xs
                        ).then_inc(dve_sem)
                        vector._custom_dve(
                            DM_COUNTER, out=cc2, in0=cc, in1=vslot
                        )
                        vector._custom_dve(
                            DM_DELTA, out=dl2, in0=cc2, in1=dl,
                            s0=-3.0, s1=0.1, imm2=0.02,
                        )
                dli = cci = KW % 2
                for t in range(KW, NSTEP):
                    if t == 16:
                        vector.wait_ge(dma_sem, 48)
                    elif t == 32:
                        vector.wait_ge(dma_sem, 64)
                    elif t == 105:
                        vector.wait_ge(dma_sem, 80)
                    xs = xv[:, :, t + 1]
                    y = ys[t % 2]
                    dl, cc = dls[dli], ccs[cci]
                    dl2, cc2 = dls[1 - dli], ccs[1 - cci]
                    vslot = xv[:, :, t]
                    vector.tensor_tensor(y[:], xs, dc[:], AluOp.subtract)
                    if t == NSTEP - 1:
                        # final step: only v is consumed; finish col 231's
                        # up/dn here so the last stores skip the Act hop.
                        vector._custom_dve(
                            DM_V, out=vslot, in0=y[:], in1=dl[:]
                        ).then_inc(dve_sem)
                        cl = CHUNKS[-1][1] - 1
                        # stage col cl is also used by chunk 7's staging:
                        # wait its stores (DMA #21/#22) before overwriting
                        vector.wait_ge(dma_sem, 16 * 21)
                        vector._custom_dve(
                            DM_UP, out=s_up[:, :, cl : cl + 1], in0=vslot
                        ).then_inc(dve_sem)
                        vector.wait_ge(dma_sem, 16 * 22)
                        vector._custom_dve(
                            DM_DN, out=s_dn[:, :, cl : cl + 1], in0=vslot
                        ).then_inc(dve_sem)
                        break
                    vector._custom_dve(DM_V, out=vslot, in0=y[:], in1=dl[:])
                    vector.copy_predicated(
                        dc[:], vslot.bitcast(mybir.dt.int32), xs
                    ).then_inc(dve_sem)
                    vector._custom_dve(
                        DM_COUNTER, out=cc2[:], in0=cc[:], in1=vslot
                    )
                    vector._custom_dve(
                        DM_DELTA, out=dl2[:], in0=cc2[:], in1=dl[:],
                        s0=-3.0, s1=0.1, imm2=0.02,
                    )
                    dli, cci = 1 - dli, 1 - cci

            @block.scalar
            def _(scalar):
                for k, (c0, cn) in enumerate(CHUNKS):
                    pieces = _pieces(k, cn)
                    for stage, scale, base, war in (
                        (s_up, 1.0, c0, 16 * (2 * k + 5)),
                        (s_dn, -1.0, NSTEP + c0, 16 * (2 * k + 6)),
                    ):
                        p0 = 0
                        first = True
                        for pn in pieces:
                            # piece covers cols [c0+p0, c0+p0+pn): needs
                            # v(c0+p0+pn-1): dve_sem >= 1+c0+p0+pn (via
                            # cp inc; the final step's DM_V carries it)
                            scalar.wait_ge(dve_sem, 6 + c0 + p0 + pn)
                            if first and k >= 1:
                                # stage WAR: previous store of this plane
                                scalar.wait_ge(dma_sem, war)
                                first = False
                            scalar.activation(
                                stage[:, :, p0 : p0 + pn],
                                xv[:, :, c0 + p0 : c0 + p0 + pn],
                                Relu, 0.0, scale,
                            ).then_inc(act_sem)
                            p0 += pn

    nc.compile()
    _NC_CACHE[key] = nc
    return nc


def kernel(x: np.ndarray) -> np.ndarray:
    x = np.ascontiguousarray(np.asarray(x), dtype=np.float32)
    assert x.shape == (B, R, C), x.shape
    nc = _build_nc()
    in_maps = [
        {"x": np.ascontiguousarray(x[c * BPC : (c + 1) * BPC].reshape(INST, C))}
        for c in range(NCORES)
    ]
    res = run_bass_kernel_spmd(
        nc,
        in_maps,
        core_ids=list(range(NCORES)),
        trace=bool(int(os.environ.get("KERNEL_TRACE", "0"))),
    )
    global LAST_RESULTS
    LAST_RESULTS = res
    outs = [r["out"].reshape(BPC, R, OUTC) for r in res.results]
    return np.concatenate(outs, axis=0)


LAST_RESULTS = None


if __name__ == "__main__":
    xs = np.random.default_rng(0).standard_normal((B, R, C), dtype=np.float32)
    o = kernel(xs)
    print(o.shape, o.dtype)


# revision 12
# speedup vs baseline: 1.0022x; 1.0022x over previous
"""Delta-modulator scan kernel for Trainium2 — raw bass (no Tile).

Per (b, r): sequential scan over the first 232 columns of x[.,.,252] with
state (dc, delta, signed run-counter); outputs UP[232] | DN[232] | x[232:252]
-> out [., ., 484] f32. Data-parallel over batch: 16 batches/core, 8 cores;
per-core 16384 instances laid out as [128 partitions x 128 free].

Structure (timings per the TimelineSim cost model; 251.0us/core total):
- The whole scan runs on the DVE back-to-back with NO semaphores inside the
  loop (in-order engine; full-width streaming ops make same-engine RAW safe):
  y = x - dc; v = (y>dl)-(y<-dl) written into xv slot t (in place over the
  consumed x column); copy_predicated dc-latch; run-counter; delta update.
  5 ops x ~194ns x 232 steps ~= 225us. Tile's per-dependency semaphore
  chains would add ~190ns/step — that is why this is raw bass.
- Warm start: the first 16 columns load as two f-half DMAs and steps 0..4
  run per-half at half width, filling DVE idle time while loads land.
- Remaining loads: [16:32) at the descriptor floor, then [32:160) and
  [104:232) as 128-column transfers whose 512B contiguous runs avoid the
  <512B 2x DMA penalty ([104:160) is rewritten with identical data).
- up/dn are extracted from the v history by the otherwise-idle Activation
  engine (relu(v), relu(-v)) into small staging tiles, in column chunks
  sized to minimize max_k(chunk readiness + remaining store mass): 32-col
  early, 24/21/18-col late, 17-col last (floor-rate pair). Extractions are
  split into pieces so each store fires as soon as its last column lands;
  column 231's up/dn are finished on the DVE right after the last DM_V
  (guarded against the staging-column reuse by chunk 7's stores).
- Tail passthrough out[464:484) = x[232:252) as a direct DRAM->DRAM DMA.
- Manual semaphores: dma_sem (+16 per DMA, FIFO order), dve_sem (+1 per
  step via copy_predicated, +1 by the init memset, +1 each by the final
  DM_V/DM_UP/DM_DN), act_sem (+1 per extraction piece).
"""

import os
from contextlib import ExitStack

import numpy as np

import concourse.bass as bass
from concourse import bacc, mybir
from concourse.bass_utils import run_bass_kernel_spmd
import concourse.dve_ops as dve_ops_mod
from concourse.dve_spec import (
    Spec, Src0, Src1, C0, C1, C2, Zero, One, maxx, minn, select, lower,
)
from concourse.dve_spec import _has_src1
from concourse.dve_uop import DveOpSpec

AluOp = mybir.AluOpType
F32 = mybir.dt.float32


def _register_op(name: str, spec: Spec) -> "dve_ops_mod.DveOp":
    """Register a custom DVE op at runtime (compute + pin its uop sha)."""
    for existing in dve_ops_mod.OPS:
        if existing.name == name:
            return existing
    opcode = dve_ops_mod._CUSTOM_DVE_ROW_BASE + len(dve_ops_mod.OPS)
    assert opcode < 0x20
    shas = {}
    for ver in ("v3",):
        tmp = DveOpSpec(
            name=name, opcode=opcode, uops=lower(spec, ver=ver), rd1_en=_has_src1(spec)
        )
        shas[ver] = tmp.sha(ver)
    op = dve_ops_mod.DveOp(name, spec, subdim=False, uops_sha=shas)
    dve_ops_mod.OPS.append(op)
    dve_ops_mod._SUB_OPCODE_FOR_NAME[name] = opcode
    dve_ops_mod.CUSTOM_DVE_SPECS[name] = spec
    return op


# cc' = trig ? max(cc,0)+1 : min(cc,0)-1   (in0=cc, in1=v; trig = in1 != 0)
DM_COUNTER = _register_op(
    "DM_COUNTER_ANT",
    Spec(
        body=select(Src1, maxx(Src0, Zero) + One, minn(Src0, Zero) - One),
        reference=lambda in0, in1, s0, s1, imm2: np.where(
            in1 != 0.0, np.maximum(in0, 0) + 1, np.minimum(in0, 0) - 1
        ).astype(np.float32),
    ),
)

# dl' = min(max(dl, (cc<=-3)*0.1), max((cc<3), 0.02))  (in0=cc, in1=dl,
# s0=-3.0, s1=0.1, imm2=0.02)
DM_DELTA = _register_op(
    "DM_DELTA_ANT",
    Spec(
        body=minn(
            maxx(Src1, (Src0 <= C0) * C1),
            maxx(Src0 < (Zero - C0), C2),
        ),
        reference=lambda in0, in1, s0, s1, imm2: np.minimum(
            np.maximum(in1, (in0 <= s0).astype(np.float32) * s1),
            np.maximum((in0 < -s0).astype(np.float32), imm2),
        ).astype(np.float32),
    ),
)

# v = (y > dl) - (y < -dl) in {-1, 0, +1} (never -0.0). Doubles as the
# predication mask (bit pattern nonzero iff trigger).  (in0=y, in1=dl)
DM_V = _register_op(
    "DM_V2_ANT",
    Spec(
        body=(Src0 > Src1) - (Src0 < (Zero - Src1)),
        reference=lambda in0, in1, s0, s1, imm2: (
            (in0 > in1).astype(np.float32) - (in0 < -in1).astype(np.float32)
        ),
    ),
)

# up = max(v, 0); dn = max(-v, 0) — single-input finishers for col 231,
# run on the DVE right after the final DM_V so the closing stores skip
# the Activation-engine round-trip.
DM_UP = _register_op(
    "DM_UP_ANT",
    Spec(
        body=maxx(Src0, Zero),
        reference=lambda in0, in1, s0, s1, imm2: np.maximum(in0, 0).astype(
            np.float32
        ),
    ),
)
DM_DN = _register_op(
    "DM_DN_ANT",
    Spec(
        body=maxx(Zero - Src0, Zero),
        reference=lambda in0, in1, s0, s1, imm2: np.maximum(-in0, 0).astype(
            np.float32
        ),
    ),
)

B, R, C = 128, 1024, 252
NSTEP = 232
NTAIL = C - NSTEP  # 20
OUTC = 2 * NSTEP + NTAIL  # 484
NCORES = 8
BPC = B // NCORES  # 16
INST = BPC * R  # 16384 instances per core
P = 128
F = INST // P  # 128

# store chunking: 32-col early, then 24/20-col late chunks. 20-col pairs
# hit the 7ns/descriptor floor at the same per-column rate as 32-col but
# carry less mass after the scan's last step, which bounds the finish.
CHUNKS = [(0, 32), (32, 32), (64, 32), (96, 32), (128, 24), (152, 24),
          (176, 21), (197, 18), (215, 17)]

_NC_CACHE = {}


def _build_nc() -> bass.Bass:
    key = "nc"
    if key in _NC_CACHE:
        return _NC_CACHE[key]
    nc = bacc.Bacc("TRN2", target_bir_lowering=False, debug=False)
    x = nc.dram_tensor("x", [INST, C], F32, kind="ExternalInput").ap()
    out = nc.dram_tensor("out", [INST, OUTC], F32, kind="ExternalOutput").ap()
    x3 = x.rearrange("(p f) c -> p f c", p=P)  # [128, 128, 252]
    o3 = out.rearrange("(p f) c -> p f c", p=P)  # [128, 128, 484]

    Relu = mybir.ActivationFunctionType.Relu
    NDMA = 6 + 2 * (len(CHUNKS) - 1) + 4

    def _pieces(k, cn):
        # split each plane's extraction so the store can fire right after
        # the chunk's LAST column's v lands. The final chunk is handled
        # explicitly per f-half (not through this helper).
        return [cn - 1, 1]

    with ExitStack() as ctx:
        # xv slot k holds x_{k-1}; v_t overwrites slot t (x_{t-1} dead).
        xv_t = ctx.enter_context(nc.sbuf_tensor("xv", [P, F, NSTEP + 1], F32))
        s_up_t = ctx.enter_context(nc.sbuf_tensor("s_up", [P, F, 32], F32))
        s_dn_t = ctx.enter_context(nc.sbuf_tensor("s_dn", [P, F, 32], F32))
        dc_t = ctx.enter_context(nc.sbuf_tensor("dc", [P, F], F32))
        dl_ts = [
            ctx.enter_context(nc.sbuf_tensor(f"dl{i}", [P, F], F32))
            for i in range(2)
        ]
        cc_ts = [
            ctx.enter_context(nc.sbuf_tensor(f"cc{i}", [P, F], F32))
            for i in range(2)
        ]
        y_ts = [
            ctx.enter_context(nc.sbuf_tensor(f"y{i}", [P, F], F32))
            for i in range(2)
        ]
        dma_sem = ctx.enter_context(nc.semaphore("dma_sem"))
        dve_sem = ctx.enter_context(nc.semaphore("dve_sem"))
        act_sem = ctx.enter_context(nc.semaphore("act_sem"))

        xv = xv_t.ap()
        s_up = s_up_t.ap()
        s_dn = s_dn_t.ap()
        dc = dc_t.ap()
        dls = [t.ap() for t in dl_ts]
        ccs = [t.ap() for t in cc_ts]
        ys = [t.ap() for t in y_ts]

        with nc.Block() as block:

            @block.sync
            def _(sync_):
                sync = sync_
                # loads: x_k -> slot k+1
                sync.dma_start(
                    xv[:, 0:64, 1:17], x3[:, 0:64, 0:16]
                ).then_inc(dma_sem, 16)
                sync.dma_start(
                    xv[:, 64:128, 1:17], x3[:, 64:128, 0:16]
                ).then_inc(dma_sem, 16)
                sync.dma_start(xv[:, :, 17:33], x3[:, :, 16:32]).then_inc(
                    dma_sem, 16
                )
                sync.dma_start(xv[:, :, 33:161], x3[:, :, 32:160]).then_inc(
                    dma_sem, 16
                )
                sync.dma_start(xv[:, :, 105:233], x3[:, :, 104:232]).then_inc(
                    dma_sem, 16
                )
                # tail passthrough DRAM->DRAM
                sync.dma_start(
                    o3[:, :, 2 * NSTEP : OUTC], x3[:, :, NSTEP:C]
                ).then_inc(dma_sem, 16)
                nact = 0
                for k, (c0, cn) in enumerate(CHUNKS[:-1]):
                    nact += len(_pieces(k, cn))
                    sync.wait_ge(act_sem, nact)
                    sync.dma_start(
                        o3[:, :, c0 : c0 + cn], s_up[:, :, 0:cn]
                    ).then_inc(dma_sem, 16)
                    nact += len(_pieces(k, cn))
                    sync.wait_ge(act_sem, nact)
                    sync.dma_start(
                        o3[:, :, NSTEP + c0 : NSTEP + c0 + cn], s_dn[:, :, 0:cn]
                    ).then_inc(dma_sem, 16)
                # final chunk per f-half. dve_sem totals: 237 after full
                # cps through step 227 + half cps 228..230 land per half:
                # A: vA=238 upA=239 dnA=240 (after 3 A-cps -> 237+3? no:
                # cps 228..230 of half A bring 234+3=237, see scalar note)
                c0, cn = CHUNKS[-1]
                for f0, f1, du, dn_, a_up, a_dn in (
                    (0, 64, 239, 240, 2, 4),
                    (64, 128, 245, 246, 6, 8),
                ):
                    sync.wait_ge(act_sem, nact + a_up)
                    sync.wait_ge(dve_sem, du)
                    sync.dma_start(
                        o3[:, f0:f1, c0 : c0 + cn], s_up[:, f0:f1, 0:cn]
                    ).then_inc(dma_sem, 16)
                    sync.wait_ge(act_sem, nact + a_dn)
                    sync.wait_ge(dve_sem, dn_)
                    sync.dma_start(
                        o3[:, f0:f1, NSTEP + c0 : NSTEP + c0 + cn],
                        s_dn[:, f0:f1, 0:cn],
                    ).then_inc(dma_sem, 16)
                sync.wait_ge(dma_sem, 16 * NDMA)

            @block.vector
            def _(vector):
                vector.memset(dc[:], 0.0)
                vector.memset(dls[0][:], 0.1)
                vector.memset(ccs[0][:], 0.0).then_inc(dve_sem)
                KW = 5  # warm-start steps run per f-half while loads land
                for f0, f1, gate in ((0, 64, 16), (64, 128, 32)):
                    vector.wait_ge(dma_sem, gate)
                    for t in range(KW):
                        xs = xv[:, f0:f1, t + 1]
                        y = ys[t % 2][:, f0:f1]
                        dl = dls[t % 2][:, f0:f1]
                        dl2 = dls[1 - t % 2][:, f0:f1]
                        cc = ccs[t % 2][:, f0:f1]
                        cc2 = ccs[1 - t % 2][:, f0:f1]
                        vslot = xv[:, f0:f1, t]
                        vector.tensor_tensor(
                            y, xs, dc[:, f0:f1], AluOp.subtract
                        )
                        vector._custom_dve(DM_V, out=vslot, in0=y, in1=dl)
                        vector.copy_predicated(
                            dc[:, f0:f1], vslot.bitcast(mybir.dt.int32), xs
                        ).then_inc(dve_sem)
                        vector._custom_dve(
                            DM_COUNTER, out=cc2, in0=cc, in1=vslot
                        )
                        vector._custom_dve(
                            DM_DELTA, out=dl2, in0=cc2, in1=dl,
                            s0=-3.0, s1=0.1, imm2=0.02,
                        )
                dli = cci = KW % 2
                KE = 228  # steps >= KE run per f-half (early final stores)
                for t in range(KW, KE):
                    if t == 16:
                        vector.wait_ge(dma_sem, 48)
                    elif t == 32:
                        vector.wait_ge(dma_sem, 64)
                    elif t == 105:
                        vector.wait_ge(dma_sem, 80)
                    xs = xv[:, :, t + 1]
                    y = ys[t % 2]
                    dl, cc = dls[dli], ccs[cci]
                    dl2, cc2 = dls[1 - dli], ccs[1 - cci]
                    vslot = xv[:, :, t]
                    vector.tensor_tensor(y[:], xs, dc[:], AluOp.subtract)
                    vector._custom_dve(DM_V, out=vslot, in0=y[:], in1=dl[:])
                    vector.copy_predicated(
                        dc[:], vslot.bitcast(mybir.dt.int32), xs
                    ).then_inc(dve_sem)
                    vector._custom_dve(
                        DM_COUNTER, out=cc2[:], in0=cc[:], in1=vslot
                    )
                    vector._custom_dve(
                        DM_DELTA, out=dl2[:], in0=cc2[:], in1=dl[:],
                        s0=-3.0, s1=0.1, imm2=0.02,
                    )
                    dli, cci = 1 - dli, 1 - cci
                # steps KE..231 per f-half; each half finishes col 231's
                # up/dn on the DVE so its stores can fire immediately.
                cl = CHUNKS[-1][1] - 1
                for hi, (f0, f1) in enumerate(((0, 64), (64, 128))):
                    hd, hc = dli, cci
                    for t in range(KE, NSTEP):
                        xs = xv[:, f0:f1, t + 1]
                        y = ys[t % 2][:, f0:f1]
                        dl = dls[hd][:, f0:f1]
                        dl2 = dls[1 - hd][:, f0:f1]
                        cc = ccs[hc][:, f0:f1]
                        cc2 = ccs[1 - hc][:, f0:f1]
                        vslot = xv[:, f0:f1, t]
                        vector.tensor_tensor(
                            y, xs, dc[:, f0:f1], AluOp.subtract
                        )
                        if t == NSTEP - 1:
                            vector._custom_dve(
                                DM_V, out=vslot, in0=y, in1=dl
                            ).then_inc(dve_sem)
                            # stage col cl also used by chunk 7's staging:
                            # wait its stores before overwriting
                            vector.wait_ge(dma_sem, 16 * 21)
                            vector._custom_dve(
                                DM_UP, out=s_up[:, f0:f1, cl : cl + 1],
                                in0=vslot,
                            ).then_inc(dve_sem)
                            vector.wait_ge(dma_sem, 16 * 22)
                            vector._custom_dve(
                                DM_DN, out=s_dn[:, f0:f1, cl : cl + 1],
                                in0=vslot,
                            ).then_inc(dve_sem)
                            break
                        vector._custom_dve(DM_V, out=vslot, in0=y, in1=dl)
                        vector.copy_predicated(
                            dc[:, f0:f1], vslot.bitcast(mybir.dt.int32), xs
                        ).then_inc(dve_sem)
                        vector._custom_dve(
                            DM_COUNTER, out=cc2, in0=cc, in1=vslot
                        )
                        vector._custom_dve(
                            DM_DELTA, out=dl2, in0=cc2, in1=dl,
                            s0=-3.0, s1=0.1, imm2=0.02,
                        )
                        hd, hc = 1 - hd, 1 - hc

            @block.scalar
            def _(scalar):
                for k, (c0, cn) in enumerate(CHUNKS[:-1]):
                    pieces = _pieces(k, cn)
                    for stage, scale, base, war in (
                        (s_up, 1.0, c0, 16 * (2 * k + 5)),
                        (s_dn, -1.0, NSTEP + c0, 16 * (2 * k + 6)),
                    ):
                        p0 = 0
                        first = True
                        for pn in pieces:
                            # piece covers cols [c0+p0, c0+p0+pn): needs
                            # v(c0+p0+pn-1): dve_sem >= 1+c0+p0+pn (via
                            # cp inc; the final step's DM_V carries it)
                            scalar.wait_ge(dve_sem, 6 + c0 + p0 + pn)
                            if first and k >= 1:
                                # stage WAR: previous store of this plane
                                scalar.wait_ge(dma_sem, war)
                                first = False
                            scalar.activation(
                                stage[:, :, p0 : p0 + pn],
                                xv[:, :, c0 + p0 : c0 + p0 + pn],
                                Relu, 0.0, scale,
                            ).then_inc(act_sem)
                            p0 += pn
                # final chunk per f-half: pieces [cn-4, 3]; cols < 228 were
                # written by full-width steps (cp through 227 -> dve=234);
                # col 228..230 by each half's cps: A done at 237, B at 243.
                c0, cn = CHUNKS[-1]
                kl = len(CHUNKS) - 1
                for f0, f1, g2 in ((0, 64, 237), (64, 128, 243)):
                    for stage, scale, war in (
                        (s_up, 1.0, 16 * (2 * kl + 5)),
                        (s_dn, -1.0, 16 * (2 * kl + 6)),
                    ):
                        scalar.wait_ge(dve_sem, 234)
                        scalar.wait_ge(dma_sem, war)
                        scalar.activation(
                            stage[:, f0:f1, 0 : cn - 4],
                            xv[:, f0:f1, c0 : c0 + cn - 4], Relu, 0.0, scale,
                        ).then_inc(act_sem)
                        scalar.wait_ge(dve_sem, g2)
                        scalar.activation(
                            stage[:, f0:f1, cn - 4 : cn - 1],
                            xv[:, f0:f1, c0 + cn - 4 : c0 + cn - 1],
                            Relu, 0.0, scale,
                        ).then_inc(act_sem)

    nc.compile()
    _NC_CACHE[key] = nc
    return nc


def kernel(x: np.ndarray) -> np.ndarray:
    x = np.ascontiguousarray(np.asarray(x), dtype=np.float32)
    assert x.shape == (B, R, C), x.shape
    nc = _build_nc()
    in_maps = [
        {"x": np.ascontiguousarray(x[c * BPC : (c + 1) * BPC].reshape(INST, C))}
        for c in range(NCORES)
    ]
    res = run_bass_kernel_spmd(
        nc,
        in_maps,
        core_ids=list(range(NCORES)),
        trace=bool(int(os.environ.get("KERNEL_TRACE", "0"))),
    )
    global LAST_RESULTS
    LAST_RESULTS = res
    outs = [r["out"].reshape(BPC, R, OUTC) for r in res.results]
    return np.concatenate(outs, axis=0)


LAST_RESULTS = None


if __name__ == "__main__":
    xs = np.random.default_rng(0).standard_normal((B, R, C), dtype=np.float32)
    o = kernel(xs)
    print(o.shape, o.dtype)
